# revision 11
# baseline (speedup 1.0000x reference)
"""Single-head causal attention (B=8, T=2048, C=1024, H=64) on 8 TRN2 NeuronCores.

Strategy: pure data parallelism — batch element b runs on core b. Each core
computes, for its [T, C] slices q_b / k_b:

    Q = q_b @ Wq ; K = k_b @ Wk ; V = k_b @ Wv          (projections)
    S = Q @ K^T / sqrt(C), causal-masked ; P = exp(S)    (no max-subtract:
    out = (P @ V) / (P @ 1)                               S is well-scaled)

Device-side layout (all matmuls bf16, fp32 PSUM accumulation):
  * Host pre-transposes q/k to [C, T] and pre-blocks them [tb, p, c, t] so
    each 512-column block arrives in one fully-contiguous DMA with
    8KB-per-partition lines, and the contraction dim (C) lands on SBUF
    partitions with zero on-chip input transposes.
  * Projections produce head-major Q^T/K^T [H, T]; K and V are projected in
    one pass with a fused [Wk | Wv] stationary operand; V natural tiles come
    from 16 PE transposes of the [K^T; V^T] blocks.
  * Scores are computed transposed (S^T[j, i] tiles, key index j on
    partitions) so P^T is directly the moving operand of the P @ V matmul —
    no P transposes. Softmax denominators come free via a ones column
    appended to V (row H of the accumulator is P @ 1).
  * exp runs on the scalar engine straight out of PSUM with 1/sqrt(C) folded
    into the activation's free scale.
  * Normalization happens in the transposed [H+1, T] layout: reciprocal of
    the l row, partition-broadcast it with a stride-0 SBUF DMA, multiply.
    The device emits out^T [H, T]; the host transposes on unshard.
  * The kernel is a single software pipeline over 512-column i-blocks:
    DMA block -> project block -> score/exp/accumulate -> normalize, store.
    Projection blocks interleave with attention blocks to keep TensorE dense
    (HAM clock gate stays at 8/8) and overlap input DMA with compute.
"""

import numpy as np
import ml_dtypes

B, T, C, H = 8, 2048, 1024, 64
P = 128                  # SBUF partitions
CCH = C // P             # 8 contraction chunks
NJ = T // P              # 16 key tiles of 128
NB = T // 512            # 4 column blocks of 512
SCALE = float(C) ** -0.5

_cached = {}


def _build():
    import concourse.bass as bass
    import concourse.mybir as mybir
    import concourse.tile as tile
    from concourse import bacc

    dt = mybir.dt
    nc = bacc.Bacc("TRN2", target_bir_lowering=False, debug=False, num_devices=B)

    # blocked inputs: [tb, p, c, t] so one DMA per 512-col block is contiguous
    qT = nc.dram_tensor("qT", [NB, P, CCH, 512], dt.bfloat16, kind="ExternalInput").ap()
    kT = nc.dram_tensor("kT", [NB, P, CCH, 512], dt.bfloat16, kind="ExternalInput").ap()
    wq = nc.dram_tensor("wq", [P, CCH, H], dt.bfloat16, kind="ExternalInput").ap()
    wkv = nc.dram_tensor("wkv", [P, CCH, 2 * H], dt.bfloat16, kind="ExternalInput").ap()
    dmask = nc.dram_tensor("dmask", [P, P], dt.bfloat16, kind="ExternalInput").ap()
    idb = nc.dram_tensor("idb", [P, P], dt.bfloat16, kind="ExternalInput").ap()
    out_t = nc.dram_tensor("out_t", [H, T], dt.float32, kind="ExternalOutput").ap()

    EXP = mybir.ActivationFunctionType.Exp

    with tile.TileContext(nc) as tc:
        with (
            tc.tile_pool(name="consts", bufs=1) as consts,
            tc.tile_pool(name="inbuf", bufs=1) as inbuf,
            tc.tile_pool(name="proj", bufs=1) as proj,
            tc.tile_pool(name="ppsum", bufs=1, space="PSUM") as ppsum,
            tc.tile_pool(name="vtpsum", bufs=1, space="PSUM") as vtpsum,
            tc.tile_pool(name="opsum", bufs=2, space="PSUM") as opsum,
            tc.tile_pool(name="spsum", bufs=3, space="PSUM") as spsum,
            tc.tile_pool(name="pbuf", bufs=6) as pbuf,
            tc.tile_pool(name="ebuf", bufs=2) as ebuf,
        ):
            # ---- constants (scalar HWDGE ring: parallel to sync ring) -------
            mask_s = consts.tile([P, P], dt.bfloat16)
            idb_s = consts.tile([P, P], dt.bfloat16)
            wq_s = consts.tile([P, CCH, H], dt.bfloat16)
            wkv_s = consts.tile([P, CCH, 2 * H], dt.bfloat16)
            nc.scalar.dma_start(out=wkv_s[:], in_=wkv[:])
            nc.scalar.dma_start(out=wq_s[:], in_=wq[:])
            nc.scalar.dma_start(out=mask_s[:], in_=dmask[:])
            nc.scalar.dma_start(out=idb_s[:], in_=idb[:])

            kT_s = inbuf.tile([P, NB, CCH, 512], dt.bfloat16)
            qT_s = inbuf.tile([P, NB, CCH, 512], dt.bfloat16)
            KVT_s = proj.tile([P, T], dt.bfloat16)   # rows 0:64 K^T, 64:128 V^T
            QT_s = proj.tile([H, T], dt.bfloat16)
            V1_s = proj.tile([P, NJ, 66], dt.bfloat16)  # V natural + ones col
            nc.vector.memset(V1_s[:, :, 64:66], 1.0)

            # ---- pipeline stages --------------------------------------------
            def proj_block(tb):
                """DMA one 512-col block of k/q and project it."""
                sl = slice(512 * tb, 512 * (tb + 1))
                nc.sync.dma_start(out=kT_s[:, tb], in_=kT[tb])
                nc.sync.dma_start(out=qT_s[:, tb], in_=qT[tb])

                KVTp = ppsum.tile([P, 512], dt.float32, tag="kvt")
                for c in range(CCH):
                    nc.tensor.matmul(KVTp[:], lhsT=wkv_s[:, c, :],
                                     rhs=kT_s[:, tb, c, :],
                                     start=(c == 0), stop=(c == CCH - 1))
                nc.vector.tensor_copy(out=KVT_s[:, sl], in_=KVTp[:])

                for jj in range(4):
                    j = 4 * tb + jj
                    vtp = vtpsum.tile([P, P], dt.bfloat16, tag="vt")
                    nc.tensor.transpose(
                        vtp[:], KVT_s[:, P * j:P * (j + 1)], idb_s[:])
                    nc.vector.tensor_copy(out=V1_s[:, j, 0:64], in_=vtp[:, 64:128])

                QTp = ppsum.tile([H, 512], dt.float32, tag="qt")
                for c in range(CCH):
                    nc.tensor.matmul(QTp[:], lhsT=wq_s[:, c, :],
                                     rhs=qT_s[:, tb, c, :],
                                     start=(c == 0), stop=(c == CCH - 1))
                nc.vector.tensor_copy(out=QT_s[:, sl], in_=QTp[:])

            def attn_block(ic):
                """Score/exp/accumulate + normalize/store one 512-col i-block."""
                ilo = 512 * ic
                OUTp = opsum.tile([H + 1, 512], dt.float32, tag="out")
                for j in range(4 * ic + 4):
                    lo = max(P * j, ilo)       # global start col of this chunk
                    n = 512 * (ic + 1) - lo    # chunk width (512 or less @diag)
                    Sp = spsum.tile([P, 512], dt.float32, tag="s")
                    nc.tensor.matmul(Sp[:, 0:n], lhsT=KVT_s[0:H, P * j:P * (j + 1)],
                                     rhs=QT_s[:, lo:lo + n], start=True, stop=True)
                    Pt = pbuf.tile([P, 512], dt.bfloat16, tag="p")
                    nc.scalar.activation(out=Pt[:, 0:n], in_=Sp[:, 0:n],
                                         func=EXP, scale=SCALE)
                    if j >= 4 * ic:
                        # diagonal block: zero strictly-upper 128x128 triangle
                        nc.vector.tensor_mul(Pt[:, 0:P], Pt[:, 0:P], mask_s[:])
                    nc.tensor.matmul(OUTp[:, lo - ilo:512], lhsT=V1_s[:, j, 0:65],
                                     rhs=Pt[:, 0:n],
                                     start=(j == 0), stop=(j == 4 * ic + 3))

                # normalize in transposed layout: rows 0:H divided by row H
                linv = ebuf.tile([1, 512], dt.float32, tag="l")
                nc.vector.reciprocal(linv[:], OUTp[H:H + 1, :])
                lbc = ebuf.tile([H, 512], dt.float32, tag="b")
                nc.gpsimd.partition_broadcast(lbc[:], linv[:])
                ot = ebuf.tile([H, 512], dt.float32, tag="o")
                nc.vector.tensor_mul(ot[:], OUTp[0:H, :], lbc[:])
                nc.sync.dma_start(out=out_t[:, ilo:ilo + 512], in_=ot[:])

            # Interleave projection blocks with attention blocks so TensorE
            # stays dense during the exp-paced attention stretches.
            proj_block(0)
            proj_block(1)
            attn_block(0)
            proj_block(2)
            attn_block(1)
            proj_block(3)
            attn_block(2)
            attn_block(3)

    nc.compile()
    return nc


def _get_nc():
    if "nc" not in _cached:
        _cached["nc"] = _build()
    return _cached["nc"]


def _block(xT):
    """[C, T] -> [NB, P, CCH, 512] so each 512-col block is contiguous."""
    return np.ascontiguousarray(
        xT.reshape(CCH, P, NB, 512).transpose(2, 1, 0, 3))


def _wblock(w):
    """[C, Hw] -> [P, CCH, Hw] contiguous (contraction chunks on partitions)."""
    return np.ascontiguousarray(
        w.reshape(CCH, P, w.shape[1]).transpose(1, 0, 2))


def _host_inputs(q, k, Wq, Wk, Wv):
    bf16 = ml_dtypes.bfloat16
    wq_h = _wblock(Wq.astype(bf16))
    wkv_h = _wblock(np.concatenate([Wk, Wv], axis=1).astype(bf16))
    dmask_h = np.triu(np.ones((P, P), dtype=np.float32)).astype(bf16)
    idb_h = np.eye(P, dtype=np.float32).astype(bf16)
    in_maps = []
    for b in range(B):
        in_maps.append({
            "qT": _block(q[b].T.astype(bf16)),
            "kT": _block(k[b].T.astype(bf16)),
            "wq": wq_h,
            "wkv": wkv_h,
            "dmask": dmask_h,
            "idb": idb_h,
        })
    return in_maps


def kernel(q, k, Wq, Wk, Wv):
    from concourse.bass_utils import run_bass_kernel_spmd

    nc = _get_nc()
    in_maps = _host_inputs(q, k, Wq, Wk, Wv)
    res = run_bass_kernel_spmd(nc, in_maps, list(range(B)))
    return np.stack(
        [res.results[b]["out_t"].T for b in range(B)]).astype(np.float32)


if __name__ == "__main__":
    rng = np.random.default_rng(0)
    q = rng.standard_normal((B, T, C)).astype(np.float32)
    k = rng.standard_normal((B, T, C)).astype(np.float32)
    Wq = (rng.standard_normal((C, H)) * 0.02).astype(np.float32)
    Wk = (rng.standard_normal((C, H)) * 0.02).astype(np.float32)
    Wv = (rng.standard_normal((C, H)) * 0.02).astype(np.float32)
    o = kernel(q, k, Wq, Wk, Wv)
    print("out", o.shape, o.dtype, float(np.abs(o).max()))


# revision 18
# speedup vs baseline: 1.1267x; 1.1267x over previous
"""Single-head causal attention (B=8, T=2048, C=1024, H=64) on 8 TRN2 NeuronCores.

Strategy: pure data parallelism — batch element b runs on core b. Each core
computes, for its [T, C] slices q_b / k_b:

    Q = q_b @ Wq ; K = k_b @ Wk ; V = k_b @ Wv          (projections)
    S = Q @ K^T / sqrt(C), causal-masked ; P = exp(S)    (no max-subtract:
    out = (P @ V) / (P @ 1)                               S is well-scaled)

Device-side layout (all matmuls bf16, fp32 PSUM accumulation):
  * Host pre-transposes q/k to [C, T] and pre-blocks them [tb, p, c, t] so
    each 512-column block arrives in one fully-contiguous DMA with
    8KB-per-partition lines, and the contraction dim (C) lands on SBUF
    partitions with zero on-chip input transposes.
  * Projections produce head-major Q^T/K^T [H, T]; K and V are projected in
    one pass with a fused [Wk | Wv] stationary operand; V natural tiles come
    from 16 PE transposes of the [K^T; V^T] blocks.
  * Scores are computed transposed (S^T[j, i] tiles, key index j on
    partitions) so P^T is directly the moving operand of the P @ V matmul —
    no P transposes. Softmax denominators come free via a ones column
    appended to V (row H of the accumulator is P @ 1).
  * exp runs on the scalar engine straight out of PSUM with 1/sqrt(C) folded
    into the activation's free scale.
  * Normalization happens in the transposed [H+1, T] layout: reciprocal of
    the l row, partition-broadcast it with a stride-0 SBUF DMA, multiply.
    The device emits out^T [H, T]; the host transposes on unshard.
  * The kernel is a single software pipeline over 512-column i-blocks:
    DMA block -> project block -> score/exp/accumulate -> normalize, store.
    Projection blocks interleave with attention blocks to keep TensorE dense
    (HAM clock gate stays at 8/8) and overlap input DMA with compute.
"""

import numpy as np
import ml_dtypes

B, T, C, H = 8, 2048, 1024, 64
P = 128                  # SBUF partitions
CCH = C // P             # 8 contraction chunks
NJ = T // P              # 16 key tiles of 128
NB = T // 512            # 4 column blocks of 512
SCALE = float(C) ** -0.5

_cached = {}


def _build():
    import concourse.bass as bass
    import concourse.mybir as mybir
    import concourse.tile as tile
    from concourse import bacc

    dt = mybir.dt
    nc = bacc.Bacc("TRN2", target_bir_lowering=False, debug=False, num_devices=B)

    # blocked inputs: [tb, p, c, t] so one DMA per 512-col block is contiguous
    qT = nc.dram_tensor("qT", [NB, P, CCH, 512], dt.bfloat16, kind="ExternalInput").ap()
    kT = nc.dram_tensor("kT", [NB, P, CCH, 512], dt.bfloat16, kind="ExternalInput").ap()
    wq = nc.dram_tensor("wq", [P, CCH, H], dt.bfloat16, kind="ExternalInput").ap()
    wkv = nc.dram_tensor("wkv", [P, CCH, 2 * H], dt.bfloat16, kind="ExternalInput").ap()
    dmask = nc.dram_tensor("dmask", [P, P], dt.bfloat16, kind="ExternalInput").ap()
    idb = nc.dram_tensor("idb", [P, P], dt.bfloat16, kind="ExternalInput").ap()
    out_t = nc.dram_tensor("out_t", [H + 1, T], dt.float32, kind="ExternalOutput").ap()

    EXP = mybir.ActivationFunctionType.Exp

    with tile.TileContext(nc) as tc:
        with (
            tc.tile_pool(name="consts", bufs=1) as consts,
            tc.tile_pool(name="inbuf", bufs=1) as inbuf,
            tc.tile_pool(name="proj", bufs=1) as proj,
            tc.tile_pool(name="ppsum", bufs=1, space="PSUM") as ppsum,
            tc.tile_pool(name="vtpsum", bufs=1, space="PSUM") as vtpsum,
            tc.tile_pool(name="opsum", bufs=2, space="PSUM") as opsum,
            tc.tile_pool(name="spsum", bufs=3, space="PSUM") as spsum,
            tc.tile_pool(name="pbuf", bufs=6) as pbuf,
            tc.tile_pool(name="ebuf", bufs=2) as ebuf,
        ):
            # ---- constants (scalar HWDGE ring: parallel to sync ring) -------
            mask_s = consts.tile([P, P], dt.bfloat16)
            idb_s = consts.tile([P, P], dt.bfloat16)
            wq_s = consts.tile([P, CCH, H], dt.bfloat16)
            wkv_s = consts.tile([P, CCH, 2 * H], dt.bfloat16)
            nc.scalar.dma_start(out=wkv_s[:], in_=wkv[:])
            nc.scalar.dma_start(out=wq_s[:], in_=wq[:])
            nc.scalar.dma_start(out=mask_s[:], in_=dmask[:])
            nc.scalar.dma_start(out=idb_s[:], in_=idb[:])

            kT_s = inbuf.tile([P, NB, CCH, 512], dt.bfloat16)
            qT_s = inbuf.tile([P, NB, CCH, 512], dt.bfloat16)
            KVT_s = proj.tile([P, T], dt.bfloat16)   # rows 0:64 K^T, 64:128 V^T
            QT_s = proj.tile([H, T], dt.bfloat16)
            V1_s = proj.tile([P, NJ, 66], dt.bfloat16)  # ones col + V natural
            nc.vector.memset(V1_s[:, :, 0:1], 1.0)

            # ---- pipeline stages --------------------------------------------
            def proj_block(tb):
                """DMA one 512-col block of k/q and project it."""
                sl = slice(512 * tb, 512 * (tb + 1))
                nc.sync.dma_start(out=kT_s[:, tb], in_=kT[tb])
                nc.sync.dma_start(out=qT_s[:, tb], in_=qT[tb])

                KVTp = ppsum.tile([P, 512], dt.float32, tag="kvt")
                for c in range(CCH):
                    nc.tensor.matmul(KVTp[:], lhsT=wkv_s[:, c, :],
                                     rhs=kT_s[:, tb, c, :],
                                     start=(c == 0), stop=(c == CCH - 1))
                nc.vector.tensor_copy(out=KVT_s[:, sl], in_=KVTp[:])

                for jj in range(4):
                    j = 4 * tb + jj
                    vtp = vtpsum.tile([P, P], dt.bfloat16, tag="vt")
                    nc.tensor.transpose(
                        vtp[:], KVT_s[:, P * j:P * (j + 1)], idb_s[:])
                    nc.vector.tensor_copy(out=V1_s[:, j, 1:65], in_=vtp[:, 64:128])

                QTp = ppsum.tile([H, 512], dt.float32, tag="qt")
                for c in range(CCH):
                    nc.tensor.matmul(QTp[:], lhsT=wq_s[:, c, :],
                                     rhs=qT_s[:, tb, c, :],
                                     start=(c == 0), stop=(c == CCH - 1))
                nc.vector.tensor_copy(out=QT_s[:, sl], in_=QTp[:])

            def attn_block(ic):
                """Score/exp/accumulate + normalize/store one 512-col i-block."""
                ilo = 512 * ic
                OUTp = opsum.tile([H + 1, 512], dt.float32, tag="out")
                for j in range(4 * ic + 4):
                    lo = max(P * j, ilo)       # global start col of this chunk
                    n = 512 * (ic + 1) - lo    # chunk width (512 or less @diag)
                    Sp = spsum.tile([P, 512], dt.float32, tag="s")
                    nc.tensor.matmul(Sp[:, 0:n], lhsT=KVT_s[0:H, P * j:P * (j + 1)],
                                     rhs=QT_s[:, lo:lo + n], start=True, stop=True)
                    Pt = pbuf.tile([P, 512], dt.bfloat16, tag="p")
                    nc.scalar.activation(out=Pt[:, 0:n], in_=Sp[:, 0:n],
                                         func=EXP, scale=SCALE)
                    if j >= 4 * ic:
                        # diagonal block: zero strictly-upper 128x128 triangle
                        nc.vector.tensor_mul(Pt[:, 0:P], Pt[:, 0:P], mask_s[:])
                    nc.tensor.matmul(OUTp[:, lo - ilo:512], lhsT=V1_s[:, j, 0:65],
                                     rhs=Pt[:, 0:n],
                                     start=(j == 0), stop=(j == 4 * ic + 3))

                # normalize in transposed layout: rows 1:H+1 divided by row 0
                # (l sits on partition 0 so the custom recip op sees it)
                linv = ebuf.tile([1, 512], dt.float32, tag="l")
                nc.vector.reciprocal_approx_fast(linv[:], OUTp[0:1, :])
                lbc = ebuf.tile([H + 1, 512], dt.float32, tag="b")
                nc.gpsimd.partition_broadcast(lbc[:], linv[:])
                ot = ebuf.tile([H + 1, 512], dt.float32, tag="o")
                nc.vector.tensor_mul(ot[:], OUTp[:], lbc[:])
                nc.sync.dma_start(out=out_t[:, ilo:ilo + 512], in_=ot[:])

            # Interleave projection blocks with attention blocks so TensorE
            # stays dense during the exp-paced attention stretches.
            proj_block(0)
            proj_block(1)
            attn_block(0)
            proj_block(2)
            attn_block(1)
            proj_block(3)
            attn_block(2)
            attn_block(3)

    nc.compile()
    return nc


def _get_nc():
    if "nc" not in _cached:
        _cached["nc"] = _build()
    return _cached["nc"]


def _block(xT):
    """[C, T] -> [NB, P, CCH, 512] so each 512-col block is contiguous."""
    return np.ascontiguousarray(
        xT.reshape(CCH, P, NB, 512).transpose(2, 1, 0, 3))


def _wblock(w):
    """[C, Hw] -> [P, CCH, Hw] contiguous (contraction chunks on partitions)."""
    return np.ascontiguousarray(
        w.reshape(CCH, P, w.shape[1]).transpose(1, 0, 2))


def _host_inputs(q, k, Wq, Wk, Wv):
    bf16 = ml_dtypes.bfloat16
    wq_h = _wblock(Wq.astype(bf16))
    wkv_h = _wblock(np.concatenate([Wk, Wv], axis=1).astype(bf16))
    dmask_h = np.triu(np.ones((P, P), dtype=np.float32)).astype(bf16)
    idb_h = np.eye(P, dtype=np.float32).astype(bf16)
    in_maps = []
    for b in range(B):
        in_maps.append({
            "qT": _block(q[b].T.astype(bf16)),
            "kT": _block(k[b].T.astype(bf16)),
            "wq": wq_h,
            "wkv": wkv_h,
            "dmask": dmask_h,
            "idb": idb_h,
        })
    return in_maps


def kernel(q, k, Wq, Wk, Wv):
    from concourse.bass_utils import run_bass_kernel_spmd

    nc = _get_nc()
    in_maps = _host_inputs(q, k, Wq, Wk, Wv)
    res = run_bass_kernel_spmd(nc, in_maps, list(range(B)))
    return np.stack(
        [res.results[b]["out_t"][1:H + 1].T for b in range(B)]).astype(np.float32)


if __name__ == "__main__":
    rng = np.random.default_rng(0)
    q = rng.standard_normal((B, T, C)).astype(np.float32)
    k = rng.standard_normal((B, T, C)).astype(np.float32)
    Wq = (rng.standard_normal((C, H)) * 0.02).astype(np.float32)
    Wk = (rng.standard_normal((C, H)) * 0.02).astype(np.float32)
    Wv = (rng.standard_normal((C, H)) * 0.02).astype(np.float32)
    o = kernel(q, k, Wq, Wk, Wv)
    print("out", o.shape, o.dtype, float(np.abs(o).max()))


# revision 20
# speedup vs baseline: 1.2280x; 1.0899x over previous
"""Single-head causal attention (B=8, T=2048, C=1024, H=64) on 8 TRN2 NeuronCores.

Strategy: pure data parallelism — batch element b runs on core b. Each core
computes, for its [T, C] slices q_b / k_b:

    Q = q_b @ Wq ; K = k_b @ Wk ; V = k_b @ Wv          (projections)
    S = Q @ K^T / sqrt(C), causal-masked ; P = exp(S)    (no max-subtract:
    out = (P @ V) / (P @ 1)                               S is well-scaled)

Device-side layout (all matmuls bf16, fp32 PSUM accumulation):
  * Host pre-transposes q/k to [C, T] and pre-blocks them [tb, p, c, t] so
    each 512-column block arrives in one fully-contiguous DMA with
    8KB-per-partition lines, and the contraction dim (C) lands on SBUF
    partitions with zero on-chip input transposes.
  * Projections produce head-major Q^T/K^T [H, T]; K and V are projected in
    one pass with a fused [Wk | Wv] stationary operand; V natural tiles come
    from 16 PE transposes of the [K^T; V^T] blocks.
  * Scores are computed transposed (S^T[j, i] tiles, key index j on
    partitions) so P^T is directly the moving operand of the P @ V matmul —
    no P transposes. Softmax denominators come free via a ones column
    appended to V (row H of the accumulator is P @ 1).
  * exp runs on the scalar engine straight out of PSUM with 1/sqrt(C) folded
    into the activation's free scale.
  * Normalization happens in the transposed [H+1, T] layout: reciprocal of
    the l row, partition-broadcast it with a stride-0 SBUF DMA, multiply.
    The device emits out^T [H, T]; the host transposes on unshard.
  * The kernel is a single software pipeline over 512-column i-blocks:
    DMA block -> project block -> score/exp/accumulate -> normalize, store.
    Projection blocks interleave with attention blocks to keep TensorE dense
    (HAM clock gate stays at 8/8) and overlap input DMA with compute.
"""

import numpy as np
import ml_dtypes

B, T, C, H = 8, 2048, 1024, 64
P = 128                  # SBUF partitions
CCH = C // P             # 8 contraction chunks
NJ = T // P              # 16 key tiles of 128
NB = T // 512            # 4 column blocks of 512
SCALE = float(C) ** -0.5

_cached = {}


def _build():
    import concourse.bass as bass
    import concourse.mybir as mybir
    import concourse.tile as tile
    from concourse import bacc

    dt = mybir.dt
    nc = bacc.Bacc("TRN2", target_bir_lowering=False, debug=False, num_devices=B)

    # blocked inputs: [tb, p, c, t] so one DMA per 512-col block is contiguous
    qT = nc.dram_tensor("qT", [NB, P, CCH, 512], dt.bfloat16, kind="ExternalInput").ap()
    kT = nc.dram_tensor("kT", [NB, P, CCH, 512], dt.bfloat16, kind="ExternalInput").ap()
    wq = nc.dram_tensor("wq", [P, CCH, H], dt.bfloat16, kind="ExternalInput").ap()
    wkv = nc.dram_tensor("wkv", [P, CCH, 2 * H], dt.bfloat16, kind="ExternalInput").ap()
    dmask = nc.dram_tensor("dmask", [P, P], dt.bfloat16, kind="ExternalInput").ap()
    idb = nc.dram_tensor("idb", [P, P], dt.bfloat16, kind="ExternalInput").ap()
    out_t = nc.dram_tensor("out_t", [H + 1, T], dt.float32, kind="ExternalOutput").ap()

    EXP = mybir.ActivationFunctionType.Exp

    with tile.TileContext(nc) as tc:
        with (
            tc.tile_pool(name="consts", bufs=1) as consts,
            tc.tile_pool(name="inbuf", bufs=1) as inbuf,
            tc.tile_pool(name="proj", bufs=1) as proj,
            tc.tile_pool(name="ppsum", bufs=1, space="PSUM") as ppsum,
            tc.tile_pool(name="vtpsum", bufs=1, space="PSUM") as vtpsum,
            tc.tile_pool(name="opsum", bufs=2, space="PSUM") as opsum,
            tc.tile_pool(name="spsum", bufs=3, space="PSUM") as spsum,
            tc.tile_pool(name="pbuf", bufs=6) as pbuf,
            tc.tile_pool(name="ebuf", bufs=2) as ebuf,
        ):
            # ---- constants (scalar HWDGE ring: parallel to sync ring) -------
            mask_s = consts.tile([P, P], dt.bfloat16)
            idb_s = consts.tile([P, P], dt.bfloat16)
            wq_s = consts.tile([P, CCH, H], dt.bfloat16)
            wkv_s = consts.tile([P, CCH, 2 * H], dt.bfloat16)
            nc.scalar.dma_start(out=wkv_s[:], in_=wkv[:])
            nc.scalar.dma_start(out=wq_s[:], in_=wq[:])
            nc.scalar.dma_start(out=mask_s[:], in_=dmask[:])
            nc.scalar.dma_start(out=idb_s[:], in_=idb[:])

            kT_s = inbuf.tile([P, NB, CCH, 512], dt.bfloat16)
            qT_s = inbuf.tile([P, NB, CCH, 512], dt.bfloat16)
            KVT_s = proj.tile([P, T], dt.bfloat16)   # rows 0:64 K^T, 64:128 V^T
            QT_s = proj.tile([H, T], dt.bfloat16)
            V1_s = proj.tile([P, NJ, 66], dt.bfloat16)  # ones col + V natural
            nc.vector.memset(V1_s[:, :, 0:1], 1.0)

            # ---- input DMAs, all upfront in arrival-priority order ----------
            # (half-block transfers; compute depends on regions, so each
            # matmul releases as soon as its half lands)
            for tb in range(NB):
                nc.sync.dma_start(out=kT_s[:, tb, 0:4], in_=kT[tb, :, 0:4])
                nc.sync.dma_start(out=kT_s[:, tb, 4:8], in_=kT[tb, :, 4:8])
                nc.sync.dma_start(out=qT_s[:, tb, 0:4], in_=qT[tb, :, 0:4])
                nc.sync.dma_start(out=qT_s[:, tb, 4:8], in_=qT[tb, :, 4:8])

            # ---- pipeline stages --------------------------------------------
            def proj_block(tb):
                """Project one 512-col block of k/q."""
                sl = slice(512 * tb, 512 * (tb + 1))
                KVTp = ppsum.tile([P, 512], dt.float32, tag="kvt")
                for c in range(CCH):
                    nc.tensor.matmul(KVTp[:], lhsT=wkv_s[:, c, :],
                                     rhs=kT_s[:, tb, c, :],
                                     start=(c == 0), stop=(c == CCH - 1))
                nc.vector.tensor_copy(out=KVT_s[:, sl], in_=KVTp[:])

                QTp = ppsum.tile([H, 512], dt.float32, tag="qt")
                for c in range(CCH):
                    nc.tensor.matmul(QTp[:], lhsT=wq_s[:, c, :],
                                     rhs=qT_s[:, tb, c, :],
                                     start=(c == 0), stop=(c == CCH - 1))
                nc.vector.tensor_copy(out=QT_s[:, sl], in_=QTp[:])

                for jj in range(4):
                    j = 4 * tb + jj
                    vtp = vtpsum.tile([P, P], dt.bfloat16, tag="vt")
                    nc.tensor.transpose(
                        vtp[:], KVT_s[:, P * j:P * (j + 1)], idb_s[:])
                    nc.vector.tensor_copy(out=V1_s[:, j, 1:65], in_=vtp[:, 64:128])

            def attn_block(ic):
                """Score/exp/accumulate + normalize/store one 512-col i-block."""
                ilo = 512 * ic
                OUTp = opsum.tile([H + 1, 512], dt.float32, tag="out")
                for j in range(4 * ic + 4):
                    lo = max(P * j, ilo)       # global start col of this chunk
                    n = 512 * (ic + 1) - lo    # chunk width (512 or less @diag)
                    Sp = spsum.tile([P, 512], dt.float32, tag="s")
                    nc.tensor.matmul(Sp[:, 0:n], lhsT=KVT_s[0:H, P * j:P * (j + 1)],
                                     rhs=QT_s[:, lo:lo + n], start=True, stop=True)
                    Pt = pbuf.tile([P, 512], dt.bfloat16, tag="p")
                    nc.scalar.activation(out=Pt[:, 0:n], in_=Sp[:, 0:n],
                                         func=EXP, scale=SCALE)
                    if j >= 4 * ic:
                        # diagonal block: zero strictly-upper 128x128 triangle
                        nc.vector.tensor_mul(Pt[:, 0:P], Pt[:, 0:P], mask_s[:])
                    nc.tensor.matmul(OUTp[:, lo - ilo:512], lhsT=V1_s[:, j, 0:65],
                                     rhs=Pt[:, 0:n],
                                     start=(j == 0), stop=(j == 4 * ic + 3))

                # normalize in transposed layout: rows 1:H+1 divided by row 0
                # (l sits on partition 0 so the custom recip op sees it)
                linv = ebuf.tile([1, 512], dt.float32, tag="l")
                nc.vector.reciprocal_approx_fast(linv[:], OUTp[0:1, :])
                lbc = ebuf.tile([H + 1, 512], dt.float32, tag="b")
                nc.gpsimd.partition_broadcast(lbc[:], linv[:])
                ot = ebuf.tile([H + 1, 512], dt.float32, tag="o")
                nc.vector.tensor_mul(ot[:], OUTp[:], lbc[:])
                nc.sync.dma_start(out=out_t[:, ilo:ilo + 512], in_=ot[:])

            # Interleave projection blocks with attention blocks so TensorE
            # stays dense during the exp-paced attention stretches. Engines
            # execute in static per-engine order, so attn(ic) is emitted right
            # after the last block it depends on (proj ic).
            for blk in range(NB):
                proj_block(blk)
                attn_block(blk)

    nc.compile()
    return nc


def _get_nc():
    if "nc" not in _cached:
        _cached["nc"] = _build()
    return _cached["nc"]


def _block(xT):
    """[C, T] -> [NB, P, CCH, 512] so each 512-col block is contiguous."""
    return np.ascontiguousarray(
        xT.reshape(CCH, P, NB, 512).transpose(2, 1, 0, 3))


def _wblock(w):
    """[C, Hw] -> [P, CCH, Hw] contiguous (contraction chunks on partitions)."""
    return np.ascontiguousarray(
        w.reshape(CCH, P, w.shape[1]).transpose(1, 0, 2))


def _host_inputs(q, k, Wq, Wk, Wv):
    bf16 = ml_dtypes.bfloat16
    wq_h = _wblock(Wq.astype(bf16))
    wkv_h = _wblock(np.concatenate([Wk, Wv], axis=1).astype(bf16))
    dmask_h = np.triu(np.ones((P, P), dtype=np.float32)).astype(bf16)
    idb_h = np.eye(P, dtype=np.float32).astype(bf16)
    in_maps = []
    for b in range(B):
        in_maps.append({
            "qT": _block(q[b].T.astype(bf16)),
            "kT": _block(k[b].T.astype(bf16)),
            "wq": wq_h,
            "wkv": wkv_h,
            "dmask": dmask_h,
            "idb": idb_h,
        })
    return in_maps


def kernel(q, k, Wq, Wk, Wv):
    from concourse.bass_utils import run_bass_kernel_spmd

    nc = _get_nc()
    in_maps = _host_inputs(q, k, Wq, Wk, Wv)
    res = run_bass_kernel_spmd(nc, in_maps, list(range(B)))
    return np.stack(
        [res.results[b]["out_t"][1:H + 1].T for b in range(B)]).astype(np.float32)


if __name__ == "__main__":
    rng = np.random.default_rng(0)
    q = rng.standard_normal((B, T, C)).astype(np.float32)
    k = rng.standard_normal((B, T, C)).astype(np.float32)
    Wq = (rng.standard_normal((C, H)) * 0.02).astype(np.float32)
    Wk = (rng.standard_normal((C, H)) * 0.02).astype(np.float32)
    Wv = (rng.standard_normal((C, H)) * 0.02).astype(np.float32)
    o = kernel(q, k, Wq, Wk, Wv)
    print("out", o.shape, o.dtype, float(np.abs(o).max()))
